# revision 4
# baseline (speedup 1.0000x reference)
"""Trainium2 Bass kernel for nn_Decoder: 6-layer pre-LN transformer decoder.

Strategy: data-parallel over batch (B=16 -> 2 sequences per core, 8 cores).
Each core runs the full decoder on its 2 sequences; no collectives.

On-device layout is feature-major ("transposed"): activations [D_part, tok].
All matmuls in fp32r (full PE rate at N>=256, ~2^-11 input rounding).

Self-contained: hardcodes all shapes; host does only indexing/transpose/
constant prep (embedding gather, posenc table, pad-bias tables).
"""
import math
import numpy as np

B, TD, TE, VOC, D, DFF, NL, H = 16, 256, 1024, 5000, 512, 2048, 6, 8
DH = D // H          # 64
NCORES = 8
BL = B // NCORES     # 2 sequences per core
TOK = BL * TD        # 512 decoder tokens per core
ETOK = BL * TE       # 2048 encoder tokens per core
P = 128
NF = D // P          # 4 feature tiles
NEG = -30000.0       # additive mask; exp underflows to exactly 0 in fp32

_CACHE = {}


def build_nc(n_layers=NL):
    import concourse.bass as bass
    import concourse.mybir as mybir
    import concourse.tile as tile
    from concourse import bacc
    from contextlib import ExitStack

    F32 = mybir.dt.float32
    F32R = mybir.dt.float32r
    AF = mybir.ActivationFunctionType
    OP = mybir.AluOpType

    nc = bacc.Bacc("TRN2", target_bir_lowering=False, debug=False)

    def din(name, shape):
        return nc.dram_tensor(name, shape, F32, kind="ExternalInput").ap()

    x0t_d = din("x0t", [D, TOK])
    pet_d = din("pet", [D, TOK])
    enct_d = din("enct", [D, ETOK])
    selfpad_d = din("selfpad", [BL * 2, P])
    crosspad_d = din("crosspad", [BL * 8, P])
    mask01_d = din("mask01t", [P, P])
    ones_d = din("ones128", [P, P])
    win_d = din("w_in", [D, D])
    bin_d = din("b_in", [D])
    lng_d = din("lng", [3, NL, D])
    lnb_d = din("lnb", [3, NL, D])
    wq1_d = din("wq1", [NL, D, D]); wk1_d = din("wk1", [NL, D, D])
    wv1_d = din("wv1", [NL, D, D]); wo1_d = din("wo1", [NL, D, D])
    wq2_d = din("wq2", [NL, D, D]); wk2_d = din("wk2", [NL, D, D])
    wv2_d = din("wv2", [NL, D, D]); wo2_d = din("wo2", [NL, D, D])
    bq1_d = din("bq1", [NL, D]); bk1_d = din("bk1", [NL, D])
    bv1_d = din("bv1", [NL, D]); bo1_d = din("bo1", [NL, D])
    bq2_d = din("bq2", [NL, D]); bk2_d = din("bk2", [NL, D])
    bv2_d = din("bv2", [NL, D]); bo2_d = din("bo2", [NL, D])
    wff1_d = din("wff1", [NL, D, DFF]); bff1_d = din("bff1", [NL, DFF])
    wff2_d = din("wff2", [NL, DFF, D]); bff2_d = din("bff2", [NL, D])
    outt_d = nc.dram_tensor("outt", [D, TOK], F32, kind="ExternalOutput").ap()

    def r32(ap):
        return ap.bitcast(F32R)

    def vec_slice_ap(dram_ap, n):
        """[n*128] dram vector -> [128, n] sbuf layout (sb[p,c] = v[c*128+p])"""
        return bass.AP(tensor=dram_ap.tensor, offset=dram_ap.offset,
                       ap=[[1, P], [P, n]])

    def bcast_ap(dram_ap, n):
        """[n] dram vector -> [128, n] partition-broadcast"""
        return bass.AP(tensor=dram_ap.tensor, offset=dram_ap.offset,
                       ap=[[0, P], [1, n]])

    ctx = ExitStack()
    with tile.TileContext(nc) as tc:
        pers = ctx.enter_context(tc.tile_pool(name="pers", bufs=1))
        wp = ctx.enter_context(tc.tile_pool(name="wp", bufs=8))
        wop = ctx.enter_context(tc.tile_pool(name="wop", bufs=3))
        actp = ctx.enter_context(tc.tile_pool(name="actp", bufs=1))
        epool = ctx.enter_context(tc.tile_pool(name="epool", bufs=4))
        sqp = ctx.enter_context(tc.tile_pool(name="sqp", bufs=2))
        stp = ctx.enter_context(tc.tile_pool(name="stp", bufs=1))
        recp = ctx.enter_context(tc.tile_pool(name="recp", bufs=2))
        rbp = ctx.enter_context(tc.tile_pool(name="rbp", bufs=2))
        bbp = ctx.enter_context(tc.tile_pool(name="bbp", bufs=2))
        bsp = ctx.enter_context(tc.tile_pool(name="bsp", bufs=3))
        encp = ctx.enter_context(tc.tile_pool(name="encp", bufs=8))
        k2p = ctx.enter_context(tc.tile_pool(name="k2p", bufs=2))
        v2p = ctx.enter_context(tc.tile_pool(name="v2p", bufs=2))
        relup = ctx.enter_context(tc.tile_pool(name="relup", bufs=1))
        dap = ctx.enter_context(tc.tile_pool(name="dap", bufs=1))
        cap = ctx.enter_context(tc.tile_pool(name="cap", bufs=1))
        psmm = ctx.enter_context(tc.tile_pool(name="psmm", bufs=3, space="PSUM"))
        psacc = ctx.enter_context(tc.tile_pool(name="psacc", bufs=2, space="PSUM"))
        psctx = ctx.enter_context(tc.tile_pool(name="psctx", bufs=3, space="PSUM"))

        # ---- persistent constants ----
        ones128 = pers.tile([P, P], F32R, tag="ones128")
        nc.sync.dma_start(out=ones128, in_=r32(ones_d))
        mask01 = pers.tile([P, P], F32R, tag="mask01")
        nc.sync.dma_start(out=mask01, in_=r32(mask01_d))
        selfpad = pers.tile([P, BL * 2], F32, tag="selfpad")
        nc.sync.dma_start(out=selfpad, in_=bass.AP(
            tensor=selfpad_d.tensor, offset=selfpad_d.offset,
            ap=[[1, P], [P, BL * 2]]))
        crosspad = pers.tile([P, BL * 8], F32, tag="crosspad")
        nc.sync.dma_start(out=crosspad, in_=bass.AP(
            tensor=crosspad_d.tensor, offset=crosspad_d.offset,
            ap=[[1, P], [P, BL * 8]]))

        xT = [pers.tile([P, TOK], F32R, tag=f"x{f}", name=f"x{f}")
              for f in range(NF)]
        eps_sb = pers.tile([P, 1], F32, tag="eps")
        nc.vector.memset(eps_sb, 1e-5)

        # ---- input stage: x = (emb_gather + posenc) @ W_in + b_in ----
        xp = []
        for f in range(NF):
            t0 = wp.tile([P, TOK], F32R, tag="w")
            nc.sync.dma_start(out=t0, in_=r32(x0t_d[f * P:(f + 1) * P, :]))
            t1 = wp.tile([P, TOK], F32R, tag="w")
            nc.sync.dma_start(out=t1, in_=r32(pet_d[f * P:(f + 1) * P, :]))
            xi = actp.tile([P, TOK], F32R, tag=f"q{f}")
            nc.vector.tensor_add(xi, t0, t1)
            xp.append(xi)
        bin_sb = bsp.tile([P, NF], F32, tag="bias4")
        nc.sync.dma_start(out=bin_sb, in_=vec_slice_ap(bin_d, NF))
        wt = []
        for f in range(NF):
            w = wp.tile([P, D], F32R, tag="w")
            nc.sync.dma_start(out=w, in_=r32(win_d[f * P:(f + 1) * P, :]))
            wt.append(w)
        for mt in range(NF):
            ps = psmm.tile([P, TOK], F32, tag="mm")
            for kf in range(NF):
                nc.tensor.matmul(ps, wt[kf][:, mt * P:(mt + 1) * P], xp[kf],
                                 start=(kf == 0), stop=(kf == NF - 1))
            nc.vector.tensor_scalar(xT[mt], ps, bin_sb[:, mt:mt + 1], None,
                                    OP.add)

        # ---- helpers ----
        def layernorm(i_ln, l):
            g_sb = bsp.tile([P, NF], F32, tag="g4")
            nc.sync.dma_start(out=g_sb, in_=vec_slice_ap(lng_d[i_ln, l], NF))
            b_sb = bsp.tile([P, NF], F32, tag="b4")
            nc.sync.dma_start(out=b_sb, in_=vec_slice_ap(lnb_d[i_ln, l], NF))
            psm = psmm.tile([P, TOK], F32, tag="mm")
            for f in range(NF):
                nc.tensor.matmul(psm, ones128, xT[f],
                                 start=(f == 0), stop=(f == NF - 1))
            psv = psmm.tile([P, TOK], F32, tag="mm")
            for f in range(NF):
                sq = sqp.tile([P, TOK], F32R, tag="sq")
                nc.scalar.activation(sq, xT[f], AF.Square)
                nc.tensor.matmul(psv, ones128, sq,
                                 start=(f == 0), stop=(f == NF - 1))
            mean = stp.tile([P, TOK], F32, tag="mean")
            nc.vector.tensor_scalar(mean, psm, 1.0 / D, None, OP.mult)
            msq = stp.tile([P, TOK], F32, tag="msq")
            nc.scalar.activation(msq, mean, AF.Square)
            var = stp.tile([P, TOK], F32, tag="var")
            nc.vector.tensor_scalar(var, psv, 1.0 / D, None, OP.mult)
            nc.vector.tensor_tensor(var, var, msq, op=OP.subtract)
            nc.scalar.activation(var, var, AF.Sqrt, bias=eps_sb)
            rstd = stp.tile([P, TOK], F32, tag="rstd")
            nc.vector.reciprocal(rstd, var)
            outs = []
            for f in range(NF):
                tmp = stp.tile([P, TOK], F32, tag="msq")
                nc.vector.tensor_tensor(tmp, xT[f], mean, op=OP.subtract)
                nc.vector.tensor_mul(tmp, tmp, rstd)
                o = actp.tile([P, TOK], F32R, tag=f"ln{f}")
                nc.vector.tensor_scalar(o, tmp, g_sb[:, f:f + 1],
                                        b_sb[:, f:f + 1], OP.mult, OP.add)
                outs.append(o)
            return outs

        def proj_fm(w_l, b_l, src, pfx):
            """feature-major out: [mt] tiles [128, TOK], out = W.T-block path"""
            wts = []
            for kf in range(NF):
                w = wp.tile([P, D], F32R, tag="w")
                nc.sync.dma_start(out=w, in_=r32(w_l[kf * P:(kf + 1) * P, :]))
                wts.append(w)
            b_sb = bsp.tile([P, NF], F32, tag="bias4")
            nc.sync.dma_start(out=b_sb, in_=vec_slice_ap(b_l, NF))
            outs = []
            for mt in range(NF):
                ps = psmm.tile([P, TOK], F32, tag="mm")
                for kf in range(NF):
                    nc.tensor.matmul(ps, wts[kf][:, mt * P:(mt + 1) * P],
                                     src[kf], start=(kf == 0),
                                     stop=(kf == NF - 1))
                o = actp.tile([P, TOK], F32R, tag=f"{pfx}{mt}")
                nc.scalar.activation(o, ps, AF.Identity,
                                     bias=b_sb[:, mt:mt + 1])
                outs.append(o)
            return outs, wts

        def wo_residual(wo_l, bo_l, ctxT):
            """x += ctx @ Wo + bo, 2 passes of 2 column-halves, K=64 per head"""
            bo_sb = bsp.tile([P, NF], F32, tag="bias4")
            nc.sync.dma_start(out=bo_sb, in_=vec_slice_ap(bo_l, NF))
            for half in range(2):
                accs = [psacc.tile([P, TOK], F32, tag="acc", name="acc")
                        for _ in range(2)]
                for h in range(H):
                    wo_t = wop.tile([DH, 256], F32R, tag="wo")
                    nc.sync.dma_start(
                        out=wo_t,
                        in_=r32(wo_l[h * DH:(h + 1) * DH,
                                     half * 256:(half + 1) * 256]))
                    for j in range(2):
                        nc.tensor.matmul(accs[j], wo_t[:, j * P:(j + 1) * P],
                                         ctxT[h], start=(h == 0),
                                         stop=(h == H - 1))
                for j in range(2):
                    mt = half * 2 + j
                    tmp = stp.tile([P, TOK], F32, tag="msq")
                    nc.vector.tensor_scalar(tmp, accs[j],
                                            bo_sb[:, mt:mt + 1], None, OP.add)
                    nc.vector.tensor_add(xT[mt], xT[mt], tmp)

        # ---- layers ----
        for l in range(n_layers):
            # ======== self-attention ========
            ln1 = layernorm(0, l)
            qT, _ = proj_fm(wq1_d[l], bq1_d[l], ln1, "q")
            kT, _ = proj_fm(wk1_d[l], bk1_d[l], ln1, "k")
            # V token-major [tok_tile, D] per (b, tt)
            wv_t = []
            for kf in range(NF):
                w = wp.tile([P, D], F32R, tag="w")
                nc.sync.dma_start(out=w,
                                  in_=r32(wv1_d[l, kf * P:(kf + 1) * P, :]))
                wv_t.append(w)
            bv_bc = bbp.tile([P, D], F32, tag="bb")
            nc.gpsimd.dma_start(out=bv_bc, in_=bcast_ap(bv1_d[l], D))
            v_sb = []
            for bt in range(NF):      # bt = b*2 + tt
                ps = psmm.tile([P, D], F32, tag="mm")
                for kf in range(NF):
                    nc.tensor.matmul(
                        ps, ln1[kf][:, bt * P:(bt + 1) * P], wv_t[kf],
                        start=(kf == 0), stop=(kf == NF - 1))
                v = actp.tile([P, D], F32R, tag=f"v{bt}")
                nc.vector.tensor_tensor(v, ps, bv_bc, op=OP.add)
                v_sb.append(v)
            ctxT = [actp.tile([DH, TOK], F32R, tag=f"ctx{h}", name=f"ctx{h}")
                    for h in range(H)]
            for b in range(BL):
                qc = b * TD
                for hp in range(H // 2):
                    den = psctx.tile([1, 512], F32, tag="ctx")
                    cps = [psctx.tile([DH, 256], F32, tag="ctx", name="cps")
                           for _ in range(2)]
                    for kt in range(2):
                        kc = b * TD + kt * P
                        sc0 = psmm.tile([P, 256], F32, tag="mm")
                        sc1 = psmm.tile([P, 256], F32, tag="mm")
                        nc.tensor.matmul(sc0, kT[hp][0:DH, kc:kc + P],
                                         qT[hp][0:DH, qc:qc + TD],
                                         start=True, stop=True)
                        nc.tensor.matmul(sc1, kT[hp][DH:P, kc:kc + P],
                                         qT[hp][DH:P, qc:qc + TD],
                                         start=True, stop=True)
                        e = epool.tile([P, 512], F32R, tag="e")
                        pad = selfpad[:, b * 2 + kt:b * 2 + kt + 1]
                        nc.scalar.activation(e[:, 0:256], sc0, AF.Exp,
                                             bias=pad, scale=0.125)
                        nc.scalar.activation(e[:, 256:512], sc1, AF.Exp,
                                             bias=pad, scale=0.125)
                        for hh in range(2):
                            off = hh * 256
                            if kt == 0:
                                nc.vector.tensor_mul(
                                    e[:, off:off + P], e[:, off:off + P],
                                    mask01)
                            else:
                                nc.vector.tensor_scalar(
                                    e[:, off:off + P], e[:, off:off + P],
                                    0.0, None, OP.mult)
                                nc.vector.tensor_mul(
                                    e[:, off + P:off + 256],
                                    e[:, off + P:off + 256], mask01)
                        nc.tensor.matmul(den, ones128[:, 0:1], e,
                                         start=(kt == 0), stop=(kt == 1))
                        for hh in range(2):
                            h = hp * 2 + hh
                            nc.tensor.matmul(
                                cps[hh],
                                v_sb[b * 2 + kt][:, h * DH:(h + 1) * DH],
                                e[:, hh * 256:(hh + 1) * 256],
                                start=(kt == 0), stop=(kt == 1))
                    rec = recp.tile([1, 512], F32, tag="rec")
                    nc.vector.reciprocal(rec, den)
                    rb = rbp.tile([DH, 512], F32, tag="rb")
                    nc.gpsimd.partition_broadcast(rb, rec)
                    for hh in range(2):
                        h = hp * 2 + hh
                        nc.vector.tensor_mul(
                            ctxT[h][:, qc:qc + TD], cps[hh],
                            rb[:, hh * 256:(hh + 1) * 256])
            wo_residual(wo1_d[l], bo1_d[l], ctxT)

            # ======== cross-attention ========
            ln2 = layernorm(1, l)
            qT, _ = proj_fm(wq2_d[l], bq2_d[l], ln2, "q")
            wk2_t = []
            for kf in range(NF):
                w = wp.tile([P, D], F32R, tag="w")
                nc.sync.dma_start(out=w,
                                  in_=r32(wk2_d[l, kf * P:(kf + 1) * P, :]))
                wk2_t.append(w)
            wv2_t = []
            for kf in range(NF):
                w = wp.tile([P, D], F32R, tag="w")
                nc.sync.dma_start(out=w,
                                  in_=r32(wv2_d[l, kf * P:(kf + 1) * P, :]))
                wv2_t.append(w)
            bk2_sb = bsp.tile([P, NF], F32, tag="bias4")
            nc.sync.dma_start(out=bk2_sb, in_=vec_slice_ap(bk2_d[l], NF))
            bv2_bc = bbp.tile([P, D], F32, tag="bb")
            nc.gpsimd.dma_start(out=bv2_bc, in_=bcast_ap(bv2_d[l], D))
            den_acc = [dap.tile([1, 512], F32, tag=f"da{i}", name=f"da{i}")
                       for i in range(H // 2)]
            ctx_acc = [cap.tile([DH, 256], F32, tag=f"ca{h}", name=f"ca{h}")
                       for h in range(H)]
            ctxT = [actp.tile([DH, TOK], F32R, tag=f"ctx{h}", name=f"ctx{h}")
                    for h in range(H)]
            for b in range(BL):
                qc = b * TD
                for ktp in range(4):
                    t0 = b * TE + ktp * 256
                    encc = []
                    for kf in range(NF):
                        t = encp.tile([P, 256], F32R, tag="enc")
                        nc.sync.dma_start(
                            out=t,
                            in_=r32(enct_d[kf * P:(kf + 1) * P, t0:t0 + 256]))
                        encc.append(t)
                    k2c = []
                    for m in range(NF):
                        ps = psmm.tile([P, 256], F32, tag="mm")
                        for kf in range(NF):
                            nc.tensor.matmul(
                                ps, wk2_t[kf][:, m * P:(m + 1) * P], encc[kf],
                                start=(kf == 0), stop=(kf == NF - 1))
                        kc = k2p.tile([P, 256], F32R, tag=f"k2_{m}")
                        nc.scalar.activation(kc, ps, AF.Identity,
                                             bias=bk2_sb[:, m:m + 1])
                        k2c.append(kc)
                    v2c = []
                    for tt in range(2):
                        ps = psmm.tile([P, D], F32, tag="mm")
                        for kf in range(NF):
                            nc.tensor.matmul(
                                ps, encc[kf][:, tt * P:(tt + 1) * P],
                                wv2_t[kf], start=(kf == 0),
                                stop=(kf == NF - 1))
                        vc = v2p.tile([P, D], F32R, tag=f"v2_{tt}")
                        nc.vector.tensor_tensor(vc, ps, bv2_bc, op=OP.add)
                        v2c.append(vc)
                    for hp in range(H // 2):
                        for tt in range(2):
                            kt = ktp * 2 + tt
                            sc0 = psmm.tile([P, 256], F32, tag="mm")
                            sc1 = psmm.tile([P, 256], F32, tag="mm")
                            nc.tensor.matmul(
                                sc0, k2c[hp][0:DH, tt * P:(tt + 1) * P],
                                qT[hp][0:DH, qc:qc + TD],
                                start=True, stop=True)
                            nc.tensor.matmul(
                                sc1, k2c[hp][DH:P, tt * P:(tt + 1) * P],
                                qT[hp][DH:P, qc:qc + TD],
                                start=True, stop=True)
                            e = epool.tile([P, 512], F32R, tag="e")
                            pad = crosspad[:, b * 8 + kt:b * 8 + kt + 1]
                            nc.scalar.activation(e[:, 0:256], sc0, AF.Exp,
                                                 bias=pad, scale=0.125)
                            nc.scalar.activation(e[:, 256:512], sc1, AF.Exp,
                                                 bias=pad, scale=0.125)
                            dps = psctx.tile([1, 512], F32, tag="ctx")
                            nc.tensor.matmul(dps, ones128[:, 0:1], e,
                                             start=True, stop=True)
                            if kt == 0:
                                nc.vector.tensor_copy(den_acc[hp], dps)
                            else:
                                nc.vector.tensor_add(den_acc[hp],
                                                     den_acc[hp], dps)
                            for hh in range(2):
                                h = hp * 2 + hh
                                cp = psctx.tile([DH, 256], F32, tag="ctx")
                                nc.tensor.matmul(
                                    cp, v2c[tt][:, h * DH:(h + 1) * DH],
                                    e[:, hh * 256:(hh + 1) * 256],
                                    start=True, stop=True)
                                if kt == 0:
                                    nc.vector.tensor_copy(ctx_acc[h], cp)
                                else:
                                    nc.vector.tensor_add(ctx_acc[h],
                                                         ctx_acc[h], cp)
                for hp in range(H // 2):
                    rec = recp.tile([1, 512], F32, tag="rec")
                    nc.vector.reciprocal(rec, den_acc[hp])
                    rb = rbp.tile([DH, 512], F32, tag="rb")
                    nc.gpsimd.partition_broadcast(rb, rec)
                    for hh in range(2):
                        h = hp * 2 + hh
                        nc.vector.tensor_mul(
                            ctxT[h][:, qc:qc + TD], ctx_acc[h],
                            rb[:, hh * 256:(hh + 1) * 256])
            wo_residual(wo2_d[l], bo2_d[l], ctxT)

            # ======== FFN ========
            ln3 = layernorm(2, l)
            bff1_sb = bsp.tile([P, DFF // P], F32, tag="bias16")
            nc.sync.dma_start(out=bff1_sb, in_=vec_slice_ap(bff1_d[l], DFF // P))
            relu_t = []
            for mc in range(4):
                wts = []
                for kf in range(NF):
                    w = wp.tile([P, 512], F32R, tag="w")
                    nc.sync.dma_start(
                        out=w, in_=r32(wff1_d[l, kf * P:(kf + 1) * P,
                                              mc * 512:(mc + 1) * 512]))
                    wts.append(w)
                for mi in range(4):
                    ps = psmm.tile([P, TOK], F32, tag="mm")
                    for kf in range(NF):
                        nc.tensor.matmul(ps, wts[kf][:, mi * P:(mi + 1) * P],
                                         ln3[kf], start=(kf == 0),
                                         stop=(kf == NF - 1))
                    idx = mc * 4 + mi
                    rt = relup.tile([P, TOK], F32R, tag=f"relu{idx}")
                    nc.scalar.activation(rt, ps, AF.Relu,
                                         bias=bff1_sb[:, idx:idx + 1])
                    relu_t.append(rt)
            bff2_sb = bsp.tile([P, NF], F32, tag="bias4")
            nc.sync.dma_start(out=bff2_sb, in_=vec_slice_ap(bff2_d[l], NF))
            for half in range(2):
                accs = [psacc.tile([P, TOK], F32, tag="acc", name="acc")
                        for _ in range(2)]
                for kt in range(DFF // P):
                    w2 = wp.tile([P, 256], F32R, tag="w")
                    nc.sync.dma_start(
                        out=w2, in_=r32(wff2_d[l, kt * P:(kt + 1) * P,
                                               half * 256:(half + 1) * 256]))
                    for j in range(2):
                        nc.tensor.matmul(accs[j], w2[:, j * P:(j + 1) * P],
                                         relu_t[kt], start=(kt == 0),
                                         stop=(kt == DFF // P - 1))
                for j in range(2):
                    mt = half * 2 + j
                    tmp = stp.tile([P, TOK], F32, tag="msq")
                    nc.vector.tensor_scalar(tmp, accs[j],
                                            bff2_sb[:, mt:mt + 1], None,
                                            OP.add)
                    nc.vector.tensor_add(xT[mt], xT[mt], tmp)

        # ---- output ----
        for f in range(NF):
            nc.sync.dma_start(out=outt_d[f * P:(f + 1) * P, :],
                              in_=xT[f].bitcast(F32))
        ctx.close()

    nc.compile()
    return nc


# ----------------------------------------------------------------------------
# Host side
# ----------------------------------------------------------------------------

class _Runner:
    """Reusable 8-core SPMD executor (PJRT/axon path, device-resident inputs)."""

    def __init__(self, nc, n_cores=NCORES):
        import jax
        import concourse.mybir as mybir
        from jax.sharding import Mesh, PartitionSpec
        from jax.experimental.shard_map import shard_map
        from concourse.bass2jax import (
            _bass_exec_p, install_neuronx_cc_hook, partition_id_tensor)
        self.jax = jax
        install_neuronx_cc_hook()
        self.nc = nc
        self.n_cores = n_cores
        partition_name = (nc.partition_id_tensor.name
                          if nc.partition_id_tensor else None)
        in_names, out_names, out_avals, zero_outs = [], [], [], []
        for alloc in nc.m.functions[0].allocations:
            if not isinstance(alloc, mybir.MemoryLocationSet):
                continue
            name = alloc.memorylocations[0].name
            if alloc.kind == "ExternalInput":
                if name != partition_name:
                    in_names.append(name)
            elif alloc.kind == "ExternalOutput":
                shape = tuple(alloc.tensor_shape)
                dtype = mybir.dt.np(alloc.dtype)
                out_names.append(name)
                out_avals.append(jax.core.ShapedArray(shape, dtype))
                zero_outs.append(np.zeros(shape, dtype))
        self.in_names = in_names
        self.out_names = out_names
        all_names = in_names + out_names
        if partition_name is not None:
            all_names = all_names + [partition_name]

        def _body(*args):
            operands = list(args)
            if partition_name is not None:
                operands.append(partition_id_tensor())
            outs = _bass_exec_p.bind(
                *operands, out_avals=tuple(out_avals),
                in_names=tuple(all_names), out_names=tuple(out_names),
                lowering_input_output_aliases=(),
                sim_require_finite=True, sim_require_nnan=True, nc=nc)
            return tuple(outs)

        devices = jax.devices()[:n_cores]
        self.mesh = Mesh(np.asarray(devices), ("core",))
        n_in = len(in_names) + len(zero_outs)
        self.sharded = jax.jit(
            shard_map(_body, mesh=self.mesh,
                      in_specs=(PartitionSpec("core"),) * n_in,
                      out_specs=(PartitionSpec("core"),) * len(out_names),
                      check_rep=False),
            keep_unused=True)
        self.zero_outs = zero_outs
        self.dev_in = None

    def stage_inputs(self, in_maps):
        from jax.sharding import NamedSharding, PartitionSpec
        sh = NamedSharding(self.mesh, PartitionSpec("core"))
        arrs = []
        for name in self.in_names:
            cat = np.concatenate(
                [np.ascontiguousarray(m[name]) for m in in_maps], axis=0)
            arrs.append(self.jax.device_put(cat, sh))
        for z in self.zero_outs:
            cat = np.zeros((self.n_cores * z.shape[0], *z.shape[1:]), z.dtype)
            arrs.append(self.jax.device_put(cat, sh))
        self.jax.block_until_ready(arrs)
        self.dev_in = arrs

    def run(self):
        return self.sharded(*self.dev_in)

    def results(self):
        outs = self.jax.block_until_ready(self.run())
        res = []
        for c in range(self.n_cores):
            d = {}
            for i, name in enumerate(self.out_names):
                full = np.asarray(outs[i])
                per = full.shape[0] // self.n_cores
                d[name] = full[c * per:(c + 1) * per]
            res.append(d)
        return res


def _posenc():
    pos = np.arange(TD, dtype=np.float32)[:, None]
    div = np.exp(np.arange(0, D, 2, dtype=np.float32)
                 * np.float32(-math.log(10000.0) / D))
    pe = np.zeros((TD, D), np.float32)
    pe[:, 0::2] = np.sin(pos * div)
    pe[:, 1::2] = np.cos(pos * div)
    return pe


def _prep_inputs(inputs):
    f32 = lambda k: np.asarray(inputs[k], np.float32)
    targets = np.asarray(inputs["targets"]).astype(np.int64)
    tlen = np.asarray(inputs["target_lengths"]).astype(np.int64)
    elen = np.asarray(inputs["encoder_output_lengths"]).astype(np.int64)
    enc = f32("encoder_outputs")
    emb_scaled = f32("emb") * np.float32(math.sqrt(float(D)))
    pe = _posenc()

    pet = np.concatenate([pe.T] * BL, axis=1)                # [D, TOK]
    mask01t = (np.arange(P)[:, None] <= np.arange(P)[None, :]
               ).astype(np.float32)                          # [k, q] keep
    ones128 = np.ones((P, P), np.float32)

    shared = {
        "pet": pet, "mask01t": mask01t, "ones128": ones128,
        "w_in": f32("W_in"), "b_in": f32("b_in"),
        "lng": np.stack([f32("ln1_g"), f32("ln2_g"), f32("ln3_g")]),
        "lnb": np.stack([f32("ln1_b"), f32("ln2_b"), f32("ln3_b")]),
        "wq1": f32("Wq1"), "wk1": f32("Wk1"), "wv1": f32("Wv1"),
        "wo1": f32("Wo1"), "wq2": f32("Wq2"), "wk2": f32("Wk2"),
        "wv2": f32("Wv2"), "wo2": f32("Wo2"),
        "bq1": f32("bq1"), "bk1": f32("bk1"), "bv1": f32("bv1"),
        "bo1": f32("bo1"), "bq2": f32("bq2"), "bk2": f32("bk2"),
        "bv2": f32("bv2"), "bo2": f32("bo2"),
        "wff1": f32("W_ff1"), "bff1": f32("b_ff1"),
        "wff2": f32("W_ff2"), "bff2": f32("b_ff2"),
    }

    karange = np.arange(P)
    in_maps = []
    for c in range(NCORES):
        x0t = np.empty((D, TOK), np.float32)
        enct = np.empty((D, ETOK), np.float32)
        selfpad = np.zeros((BL * 2, P), np.float32)
        crosspad = np.zeros((BL * 8, P), np.float32)
        for b in range(BL):
            g = c * BL + b
            x0t[:, b * TD:(b + 1) * TD] = emb_scaled[targets[g]].T
            enct[:, b * TE:(b + 1) * TE] = enc[g].T
            for kt in range(2):
                kpos = kt * P + karange
                selfpad[b * 2 + kt] = np.where(kpos < tlen[g], 0.0, NEG)
            for kt in range(8):
                kpos = kt * P + karange
                crosspad[b * 8 + kt] = np.where(kpos < elen[g], 0.0, NEG)
        m = dict(shared)
        m.update({"x0t": x0t, "enct": enct,
                  "selfpad": selfpad, "crosspad": crosspad})
        in_maps.append(m)
    return in_maps


def _get_runner():
    if "runner" not in _CACHE:
        nc = build_nc(NL)
        _CACHE["nc"] = nc
        _CACHE["runner"] = _Runner(nc, NCORES)
    return _CACHE["runner"]


def kernel(**inputs):
    runner = _get_runner()
    in_maps = _prep_inputs(inputs)
    runner.stage_inputs(in_maps)
    res = runner.results()
    out = np.empty((B, TD, D), np.float32)
    for c in range(NCORES):
        outt = res[c]["outt"]                    # [D, TOK]
        for b in range(BL):
            out[c * BL + b] = outt[:, b * TD:(b + 1) * TD].T
    return out


# revision 8
# speedup vs baseline: 1.1440x; 1.1440x over previous
"""Trainium2 Bass kernel for nn_Decoder: 6-layer pre-LN transformer decoder.

Strategy: data-parallel over batch (B=16 -> 2 sequences per core, 8 cores).
Each core runs the full decoder on its 2 sequences; no collectives.

On-device layout is feature-major ("transposed"): activations [D_part, tok].
All matmuls in fp32r (full PE rate at N>=256, ~2^-11 input rounding).

Self-contained: hardcodes all shapes; host does only indexing/transpose/
constant prep (embedding gather, posenc table, pad-bias tables).
"""
import math
import numpy as np

B, TD, TE, VOC, D, DFF, NL, H = 16, 256, 1024, 5000, 512, 2048, 6, 8
DH = D // H          # 64
NCORES = 8
BL = B // NCORES     # 2 sequences per core
TOK = BL * TD        # 512 decoder tokens per core
ETOK = BL * TE       # 2048 encoder tokens per core
P = 128
NF = D // P          # 4 feature tiles
NEG = -30000.0       # additive mask; exp underflows to exactly 0 in fp32

_CACHE = {}


def build_nc(n_layers=NL, flags=frozenset(), weights=None):
    import concourse.bass as bass
    import concourse.mybir as mybir
    import concourse.tile as tile
    from concourse import bacc
    from contextlib import ExitStack

    F32 = mybir.dt.float32
    F32R = mybir.dt.float32r
    AF = mybir.ActivationFunctionType
    OP = mybir.AluOpType

    nc = bacc.Bacc("TRN2", target_bir_lowering=False, debug=False)

    def din(name, shape):
        return nc.dram_tensor(name, shape, F32, kind="ExternalInput").ap()

    def dwt(name, shape):
        """weight tensor: baked into the NEFF when values are provided"""
        if weights is not None:
            data = np.ascontiguousarray(weights[name], dtype=np.float32)
            assert list(data.shape) == list(shape), (name, data.shape, shape)
            return nc.inline_tensor(data, name=name).ap()
        return din(name, shape)

    x0t_d = din("x0t", [D, TOK])
    pet_d = dwt("pet", [D, TOK])
    enct_d = din("enct", [D, ETOK])
    selfpad_d = din("selfpad", [BL * 2, P])
    crosspad_d = din("crosspad", [BL * 8, P])
    mask01_d = dwt("mask01t", [P, P])
    ones_d = dwt("ones128", [P, P])
    win_d = dwt("w_in", [D, D])
    bin_d = dwt("b_in", [D])
    if n_layers > 0:
        wq1_d = dwt("wq1", [NL, D, D]); wk1_d = dwt("wk1", [NL, D, D])
        wv1_d = dwt("wv1", [NL, D, D]); wo1_d = dwt("wo1", [NL, D, D])
        wq2_d = dwt("wq2", [NL, D, D]); wk2_d = dwt("wk2", [NL, D, D])
        wv2_d = dwt("wv2", [NL, D, D]); wo2_d = dwt("wo2", [NL, D, D])
        bq1_d = dwt("bq1", [NL, D]); bk1_d = dwt("bk1", [NL, D])
        bv1_d = dwt("bv1", [NL, D]); bo1_d = dwt("bo1", [NL, D])
        bq2_d = dwt("bq2", [NL, D]); bk2_d = dwt("bk2", [NL, D])
        bv2_d = dwt("bv2", [NL, D]); bo2_d = dwt("bo2", [NL, D])
        wff1_d = dwt("wff1", [NL, D, DFF]); bff1_d = dwt("bff1", [NL, DFF])
        wff2_d = dwt("wff2", [NL, DFF, D]); bff2_d = dwt("bff2", [NL, D])
        lng_d = dwt("lng", [3, NL, D])
        lnb_d = dwt("lnb", [3, NL, D])
    outt_d = nc.dram_tensor("outt", [D, TOK], F32, kind="ExternalOutput").ap()

    def r32(ap):
        return ap.bitcast(F32R)

    def vec_slice_ap(dram_ap, n):
        """[n*128] dram vector -> [128, n] sbuf layout (sb[p,c] = v[c*128+p])"""
        return bass.AP(tensor=dram_ap.tensor, offset=dram_ap.offset,
                       ap=[[1, P], [P, n]])

    def bcast_ap(dram_ap, n):
        """[n] dram vector -> [128, n] partition-broadcast"""
        return bass.AP(tensor=dram_ap.tensor, offset=dram_ap.offset,
                       ap=[[0, P], [1, n]])

    ctx = ExitStack()
    with tile.TileContext(nc) as tc:
        pers = ctx.enter_context(tc.tile_pool(name="pers", bufs=1))
        wp = ctx.enter_context(tc.tile_pool(name="wp", bufs=8))
        wop = ctx.enter_context(tc.tile_pool(name="wop", bufs=3))
        actp = ctx.enter_context(tc.tile_pool(name="actp", bufs=1))
        epool = ctx.enter_context(tc.tile_pool(name="epool", bufs=4))
        sqp = ctx.enter_context(tc.tile_pool(name="sqp", bufs=2))
        stp = ctx.enter_context(tc.tile_pool(name="stp", bufs=1))
        recp = ctx.enter_context(tc.tile_pool(name="recp", bufs=2))
        rbp = ctx.enter_context(tc.tile_pool(name="rbp", bufs=2))
        bbp = ctx.enter_context(tc.tile_pool(name="bbp", bufs=2))
        bsp = ctx.enter_context(tc.tile_pool(name="bsp", bufs=3))
        encp = ctx.enter_context(tc.tile_pool(name="encp", bufs=8))
        k2p = ctx.enter_context(tc.tile_pool(name="k2p", bufs=2))
        v2p = ctx.enter_context(tc.tile_pool(name="v2p", bufs=2))
        relup = ctx.enter_context(tc.tile_pool(name="relup", bufs=1))
        dap = ctx.enter_context(tc.tile_pool(name="dap", bufs=1))
        cap = ctx.enter_context(tc.tile_pool(name="cap", bufs=1))
        psmm = ctx.enter_context(tc.tile_pool(name="psmm", bufs=3, space="PSUM"))
        psacc = ctx.enter_context(tc.tile_pool(name="psacc", bufs=2, space="PSUM"))
        psctx = ctx.enter_context(tc.tile_pool(name="psctx", bufs=3, space="PSUM"))

        # ---- persistent constants ----
        ones128 = pers.tile([P, P], F32R, tag="ones128")
        nc.sync.dma_start(out=ones128, in_=r32(ones_d))
        mask01 = pers.tile([P, P], F32R, tag="mask01")
        nc.sync.dma_start(out=mask01, in_=r32(mask01_d))
        selfpad = pers.tile([P, BL * 2], F32, tag="selfpad")
        nc.sync.dma_start(out=selfpad, in_=bass.AP(
            tensor=selfpad_d.tensor, offset=selfpad_d.offset,
            ap=[[1, P], [P, BL * 2]]))
        crosspad = pers.tile([P, BL * 8], F32, tag="crosspad")
        nc.sync.dma_start(out=crosspad, in_=bass.AP(
            tensor=crosspad_d.tensor, offset=crosspad_d.offset,
            ap=[[1, P], [P, BL * 8]]))

        xT = [pers.tile([P, TOK], F32R, tag=f"x{f}", name=f"x{f}")
              for f in range(NF)]
        eps_sb = pers.tile([P, 1], F32, tag="eps")
        nc.vector.memset(eps_sb, 1e-5)
        zb16 = pers.tile([P, 16], F32, tag="zb16")
        nc.vector.memset(zb16, 0.0)
        zbb = pers.tile([P, D], F32, tag="zbb")
        nc.vector.memset(zbb, 0.0)
        ones_rb = pers.tile([DH, 512], F32, tag="ones_rb")
        nc.vector.memset(ones_rb, 1.0)

        def load_vec(dram_ap, n, tag):
            if "novs" in flags:
                return zb16[:, 0:n]
            t = bsp.tile([P, n], F32, tag=tag, name=tag)
            nc.sync.dma_start(out=t, in_=vec_slice_ap(dram_ap, n))
            return t

        def load_bcast(dram_ap):
            if "nobb" in flags:
                return zbb
            t = bbp.tile([P, D], F32, tag="bb", name="bb")
            nc.gpsimd.dma_start(out=t, in_=bcast_ap(dram_ap, D))
            return t

        # ---- input stage: x = (emb_gather + posenc) @ W_in + b_in ----
        xp = []
        for f in range(NF):
            t0 = wp.tile([P, TOK], F32R, tag="w")
            nc.sync.dma_start(out=t0, in_=r32(x0t_d[f * P:(f + 1) * P, :]))
            t1 = wp.tile([P, TOK], F32R, tag="w")
            nc.sync.dma_start(out=t1, in_=r32(pet_d[f * P:(f + 1) * P, :]))
            xi = actp.tile([P, TOK], F32R, tag=f"q{f}")
            nc.vector.tensor_add(xi, t0, t1)
            xp.append(xi)
        bin_sb = load_vec(bin_d, NF, "bias4")
        wt = []
        for f in range(NF):
            w = wp.tile([P, D], F32R, tag="w")
            nc.sync.dma_start(out=w, in_=r32(win_d[f * P:(f + 1) * P, :]))
            wt.append(w)
        for mt in range(NF):
            ps = psmm.tile([P, TOK], F32, tag="mm")
            for kf in range(NF):
                nc.tensor.matmul(ps, wt[kf][:, mt * P:(mt + 1) * P], xp[kf],
                                 start=(kf == 0), stop=(kf == NF - 1))
            nc.vector.tensor_scalar(xT[mt], ps, bin_sb[:, mt:mt + 1], None,
                                    OP.add)

        # ---- helpers ----
        def layernorm(i_ln, l):
            g_sb = load_vec(lng_d[i_ln, l], NF, "g4")
            b_sb = load_vec(lnb_d[i_ln, l], NF, "b4")
            psm = psmm.tile([P, TOK], F32, tag="mm")
            for f in range(NF):
                nc.tensor.matmul(psm, ones128, xT[f],
                                 start=(f == 0), stop=(f == NF - 1))
            psv = psmm.tile([P, TOK], F32, tag="mm")
            for f in range(NF):
                sq = sqp.tile([P, TOK], F32R, tag="sq")
                nc.scalar.activation(sq, xT[f], AF.Square)
                nc.tensor.matmul(psv, ones128, sq,
                                 start=(f == 0), stop=(f == NF - 1))
            mean = stp.tile([P, TOK], F32, tag="mean")
            nc.vector.tensor_scalar(mean, psm, 1.0 / D, None, OP.mult)
            msq = stp.tile([P, TOK], F32, tag="msq")
            nc.scalar.activation(msq, mean, AF.Square)
            var = stp.tile([P, TOK], F32, tag="var")
            nc.vector.tensor_scalar(var, psv, 1.0 / D, None, OP.mult)
            nc.vector.tensor_tensor(var, var, msq, op=OP.subtract)
            nc.scalar.activation(var, var, AF.Sqrt, bias=eps_sb)
            rstd = stp.tile([P, TOK], F32, tag="rstd")
            nc.vector.reciprocal(rstd, var)
            outs = []
            for f in range(NF):
                tmp = stp.tile([P, TOK], F32, tag="msq")
                nc.vector.tensor_tensor(tmp, xT[f], mean, op=OP.subtract)
                nc.vector.tensor_mul(tmp, tmp, rstd)
                o = actp.tile([P, TOK], F32R, tag=f"ln{f}")
                nc.vector.tensor_scalar(o, tmp, g_sb[:, f:f + 1],
                                        b_sb[:, f:f + 1], OP.mult, OP.add)
                outs.append(o)
            return outs

        def proj_fm(w_l, b_l, src, pfx):
            """feature-major out: [mt] tiles [128, TOK], out = W.T-block path"""
            wts = []
            for kf in range(NF):
                w = wp.tile([P, D], F32R, tag="w")
                nc.sync.dma_start(out=w, in_=r32(w_l[kf * P:(kf + 1) * P, :]))
                wts.append(w)
            b_sb = load_vec(b_l, NF, "bias4")
            outs = []
            for mt in range(NF):
                ps = psmm.tile([P, TOK], F32, tag="mm")
                for kf in range(NF):
                    nc.tensor.matmul(ps, wts[kf][:, mt * P:(mt + 1) * P],
                                     src[kf], start=(kf == 0),
                                     stop=(kf == NF - 1))
                o = actp.tile([P, TOK], F32R, tag=f"{pfx}{mt}")
                nc.scalar.activation(o, ps, AF.Identity,
                                     bias=b_sb[:, mt:mt + 1])
                outs.append(o)
            return outs, wts

        def wo_residual(wo_l, bo_l, ctxT):
            """x += ctx @ Wo + bo, 2 passes of 2 column-halves, K=64 per head"""
            bo_sb = load_vec(bo_l, NF, "bias4")
            for half in range(2):
                accs = [psacc.tile([P, TOK], F32, tag="acc", name="acc")
                        for _ in range(2)]
                for h in range(H):
                    wo_t = wop.tile([DH, 256], F32R, tag="wo")
                    nc.sync.dma_start(
                        out=wo_t,
                        in_=r32(wo_l[h * DH:(h + 1) * DH,
                                     half * 256:(half + 1) * 256]))
                    for j in range(2):
                        nc.tensor.matmul(accs[j], wo_t[:, j * P:(j + 1) * P],
                                         ctxT[h], start=(h == 0),
                                         stop=(h == H - 1))
                for j in range(2):
                    mt = half * 2 + j
                    tmp = stp.tile([P, TOK], F32, tag="msq")
                    nc.vector.tensor_scalar(tmp, accs[j],
                                            bo_sb[:, mt:mt + 1], None, OP.add)
                    nc.vector.tensor_add(xT[mt], xT[mt], tmp)

        # ---- layers ----
        for l in range(n_layers):
            # ======== self-attention ========
            ln1 = layernorm(0, l)
            qT, _ = proj_fm(wq1_d[l], bq1_d[l], ln1, "q")
            kT, _ = proj_fm(wk1_d[l], bk1_d[l], ln1, "k")
            # V token-major [tok_tile, D] per (b, tt)
            wv_t = []
            for kf in range(NF):
                w = wp.tile([P, D], F32R, tag="w")
                nc.sync.dma_start(out=w,
                                  in_=r32(wv1_d[l, kf * P:(kf + 1) * P, :]))
                wv_t.append(w)
            bv_bc = load_bcast(bv1_d[l])
            v_sb = []
            for bt in range(NF):      # bt = b*2 + tt
                ps = psmm.tile([P, D], F32, tag="mm")
                for kf in range(NF):
                    nc.tensor.matmul(
                        ps, ln1[kf][:, bt * P:(bt + 1) * P], wv_t[kf],
                        start=(kf == 0), stop=(kf == NF - 1))
                v = actp.tile([P, D], F32R, tag=f"v{bt}")
                nc.vector.tensor_tensor(v, ps, bv_bc, op=OP.add)
                v_sb.append(v)
            ctxT = [actp.tile([DH, TOK], F32R, tag=f"ctx{h}", name=f"ctx{h}")
                    for h in range(H)]
            for b in range(BL):
                qc = b * TD
                for hp in range(H // 2):
                    den = psctx.tile([1, 512], F32, tag="ctx")
                    cps = [psctx.tile([DH, 256], F32, tag="ctx", name="cps")
                           for _ in range(2)]
                    for kt in range(2):
                        kc = b * TD + kt * P
                        sc0 = psmm.tile([P, 256], F32, tag="mm")
                        sc1 = psmm.tile([P, 256], F32, tag="mm")
                        nc.tensor.matmul(sc0, kT[hp][0:DH, kc:kc + P],
                                         qT[hp][0:DH, qc:qc + TD],
                                         start=True, stop=True)
                        nc.tensor.matmul(sc1, kT[hp][DH:P, kc:kc + P],
                                         qT[hp][DH:P, qc:qc + TD],
                                         start=True, stop=True)
                        e = epool.tile([P, 512], F32R, tag="e")
                        pad = selfpad[:, b * 2 + kt:b * 2 + kt + 1]
                        nc.scalar.activation(e[:, 0:256], sc0, AF.Exp,
                                             bias=pad, scale=0.125)
                        nc.scalar.activation(e[:, 256:512], sc1, AF.Exp,
                                             bias=pad, scale=0.125)
                        for hh in range(2):
                            off = hh * 256
                            if kt == 0:
                                nc.vector.tensor_mul(
                                    e[:, off:off + P], e[:, off:off + P],
                                    mask01)
                            else:
                                nc.vector.tensor_scalar(
                                    e[:, off:off + P], e[:, off:off + P],
                                    0.0, None, OP.mult)
                                nc.vector.tensor_mul(
                                    e[:, off + P:off + 256],
                                    e[:, off + P:off + 256], mask01)
                        if "noden" not in flags:
                            nc.tensor.matmul(den, ones128[:, 0:1], e,
                                             start=(kt == 0), stop=(kt == 1))
                        for hh in range(2):
                            h = hp * 2 + hh
                            nc.tensor.matmul(
                                cps[hh],
                                v_sb[b * 2 + kt][:, h * DH:(h + 1) * DH],
                                e[:, hh * 256:(hh + 1) * 256],
                                start=(kt == 0), stop=(kt == 1))
                    if "noden" in flags or "nopb" in flags:
                        rb = ones_rb
                        if "noden" not in flags:
                            rec = recp.tile([1, 512], F32, tag="rec")
                            nc.vector.reciprocal(rec, den)
                    else:
                        rec = recp.tile([1, 512], F32, tag="rec")
                        nc.vector.reciprocal(rec, den)
                        rb = rbp.tile([DH, 512], F32, tag="rb")
                        nc.gpsimd.partition_broadcast(rb, rec)
                    for hh in range(2):
                        h = hp * 2 + hh
                        nc.vector.tensor_mul(
                            ctxT[h][:, qc:qc + TD], cps[hh],
                            rb[:, hh * 256:(hh + 1) * 256])
            wo_residual(wo1_d[l], bo1_d[l], ctxT)

            # ======== cross-attention ========
            ln2 = layernorm(1, l)
            qT, _ = proj_fm(wq2_d[l], bq2_d[l], ln2, "q")
            wk2_t = []
            for kf in range(NF):
                w = wp.tile([P, D], F32R, tag="w")
                nc.sync.dma_start(out=w,
                                  in_=r32(wk2_d[l, kf * P:(kf + 1) * P, :]))
                wk2_t.append(w)
            wv2_t = []
            for kf in range(NF):
                w = wp.tile([P, D], F32R, tag="w")
                nc.sync.dma_start(out=w,
                                  in_=r32(wv2_d[l, kf * P:(kf + 1) * P, :]))
                wv2_t.append(w)
            bk2_sb = load_vec(bk2_d[l], NF, "bias4")
            bv2_bc = load_bcast(bv2_d[l])
            den_acc = [dap.tile([1, 512], F32, tag=f"da{i}", name=f"da{i}")
                       for i in range(H // 2)]
            ctx_acc = [cap.tile([DH, 256], F32, tag=f"ca{h}", name=f"ca{h}")
                       for h in range(H)]
            ctxT = [actp.tile([DH, TOK], F32R, tag=f"ctx{h}", name=f"ctx{h}")
                    for h in range(H)]
            for b in range(BL):
                qc = b * TD
                for ktp in range(4):
                    t0 = b * TE + ktp * 256
                    encc = []
                    for kf in range(NF):
                        t = encp.tile([P, 256], F32R, tag="enc")
                        nc.sync.dma_start(
                            out=t,
                            in_=r32(enct_d[kf * P:(kf + 1) * P, t0:t0 + 256]))
                        encc.append(t)
                    k2c = []
                    for m in range(NF):
                        ps = psmm.tile([P, 256], F32, tag="mm")
                        for kf in range(NF):
                            nc.tensor.matmul(
                                ps, wk2_t[kf][:, m * P:(m + 1) * P], encc[kf],
                                start=(kf == 0), stop=(kf == NF - 1))
                        kc = k2p.tile([P, 256], F32R, tag=f"k2_{m}")
                        nc.scalar.activation(kc, ps, AF.Identity,
                                             bias=bk2_sb[:, m:m + 1])
                        k2c.append(kc)
                    v2c = []
                    for tt in range(2):
                        ps = psmm.tile([P, D], F32, tag="mm")
                        for kf in range(NF):
                            nc.tensor.matmul(
                                ps, encc[kf][:, tt * P:(tt + 1) * P],
                                wv2_t[kf], start=(kf == 0),
                                stop=(kf == NF - 1))
                        vc = v2p.tile([P, D], F32R, tag=f"v2_{tt}")
                        nc.vector.tensor_tensor(vc, ps, bv2_bc, op=OP.add)
                        v2c.append(vc)
                    for hp in range(H // 2):
                        for tt in range(2):
                            kt = ktp * 2 + tt
                            sc0 = psmm.tile([P, 256], F32, tag="mm")
                            sc1 = psmm.tile([P, 256], F32, tag="mm")
                            nc.tensor.matmul(
                                sc0, k2c[hp][0:DH, tt * P:(tt + 1) * P],
                                qT[hp][0:DH, qc:qc + TD],
                                start=True, stop=True)
                            nc.tensor.matmul(
                                sc1, k2c[hp][DH:P, tt * P:(tt + 1) * P],
                                qT[hp][DH:P, qc:qc + TD],
                                start=True, stop=True)
                            e = epool.tile([P, 512], F32R, tag="e")
                            pad = crosspad[:, b * 8 + kt:b * 8 + kt + 1]
                            nc.scalar.activation(e[:, 0:256], sc0, AF.Exp,
                                                 bias=pad, scale=0.125)
                            nc.scalar.activation(e[:, 256:512], sc1, AF.Exp,
                                                 bias=pad, scale=0.125)
                            if "noden" not in flags:
                                dps = psctx.tile([1, 512], F32, tag="ctx")
                                nc.tensor.matmul(dps, ones128[:, 0:1], e,
                                                 start=True, stop=True)
                                if kt == 0:
                                    nc.vector.tensor_copy(den_acc[hp], dps)
                                else:
                                    nc.vector.tensor_add(den_acc[hp],
                                                         den_acc[hp], dps)
                            for hh in range(2):
                                h = hp * 2 + hh
                                cp = psctx.tile([DH, 256], F32, tag="ctx")
                                nc.tensor.matmul(
                                    cp, v2c[tt][:, h * DH:(h + 1) * DH],
                                    e[:, hh * 256:(hh + 1) * 256],
                                    start=True, stop=True)
                                if kt == 0:
                                    nc.vector.tensor_copy(ctx_acc[h], cp)
                                else:
                                    nc.vector.tensor_add(ctx_acc[h],
                                                         ctx_acc[h], cp)
                for hp in range(H // 2):
                    if "noden" in flags or "nopb" in flags:
                        rb = ones_rb
                    else:
                        rec = recp.tile([1, 512], F32, tag="rec")
                        nc.vector.reciprocal(rec, den_acc[hp])
                        rb = rbp.tile([DH, 512], F32, tag="rb")
                        nc.gpsimd.partition_broadcast(rb, rec)
                    for hh in range(2):
                        h = hp * 2 + hh
                        nc.vector.tensor_mul(
                            ctxT[h][:, qc:qc + TD], ctx_acc[h],
                            rb[:, hh * 256:(hh + 1) * 256])
            wo_residual(wo2_d[l], bo2_d[l], ctxT)

            # ======== FFN ========
            ln3 = layernorm(2, l)
            bff1_sb = load_vec(bff1_d[l], DFF // P, "bias16")
            relu_t = []
            for mc in range(4):
                wts = []
                for kf in range(NF):
                    w = wp.tile([P, 512], F32R, tag="w")
                    nc.sync.dma_start(
                        out=w, in_=r32(wff1_d[l, kf * P:(kf + 1) * P,
                                              mc * 512:(mc + 1) * 512]))
                    wts.append(w)
                for mi in range(4):
                    ps = psmm.tile([P, TOK], F32, tag="mm")
                    for kf in range(NF):
                        nc.tensor.matmul(ps, wts[kf][:, mi * P:(mi + 1) * P],
                                         ln3[kf], start=(kf == 0),
                                         stop=(kf == NF - 1))
                    idx = mc * 4 + mi
                    rt = relup.tile([P, TOK], F32R, tag=f"relu{idx}")
                    nc.scalar.activation(rt, ps, AF.Relu,
                                         bias=bff1_sb[:, idx:idx + 1])
                    relu_t.append(rt)
            bff2_sb = load_vec(bff2_d[l], NF, "bias4")
            for half in range(2):
                accs = [psacc.tile([P, TOK], F32, tag="acc", name="acc")
                        for _ in range(2)]
                for kt in range(DFF // P):
                    w2 = wp.tile([P, 256], F32R, tag="w")
                    nc.sync.dma_start(
                        out=w2, in_=r32(wff2_d[l, kt * P:(kt + 1) * P,
                                               half * 256:(half + 1) * 256]))
                    for j in range(2):
                        nc.tensor.matmul(accs[j], w2[:, j * P:(j + 1) * P],
                                         relu_t[kt], start=(kt == 0),
                                         stop=(kt == DFF // P - 1))
                for j in range(2):
                    mt = half * 2 + j
                    tmp = stp.tile([P, TOK], F32, tag="msq")
                    nc.vector.tensor_scalar(tmp, accs[j],
                                            bff2_sb[:, mt:mt + 1], None,
                                            OP.add)
                    nc.vector.tensor_add(xT[mt], xT[mt], tmp)

        # ---- output ----
        for f in range(NF):
            nc.sync.dma_start(out=outt_d[f * P:(f + 1) * P, :],
                              in_=xT[f].bitcast(F32))
        ctx.close()

    nc.compile()
    return nc


# ----------------------------------------------------------------------------
# Host side
# ----------------------------------------------------------------------------

class _Runner:
    """Reusable 8-core SPMD executor (PJRT/axon path, device-resident inputs)."""

    def __init__(self, nc, n_cores=NCORES):
        import jax
        import concourse.mybir as mybir
        from jax.sharding import Mesh, PartitionSpec
        from jax.experimental.shard_map import shard_map
        from concourse.bass2jax import (
            _bass_exec_p, install_neuronx_cc_hook, partition_id_tensor)
        self.jax = jax
        install_neuronx_cc_hook()
        self.nc = nc
        self.n_cores = n_cores
        partition_name = (nc.partition_id_tensor.name
                          if nc.partition_id_tensor else None)
        in_names, out_names, out_avals, zero_outs = [], [], [], []
        for alloc in nc.m.functions[0].allocations:
            if not isinstance(alloc, mybir.MemoryLocationSet):
                continue
            name = alloc.memorylocations[0].name
            if alloc.kind == "ExternalInput":
                if name != partition_name:
                    in_names.append(name)
            elif alloc.kind == "ExternalOutput":
                shape = tuple(alloc.tensor_shape)
                dtype = mybir.dt.np(alloc.dtype)
                out_names.append(name)
                out_avals.append(jax.core.ShapedArray(shape, dtype))
                zero_outs.append(np.zeros(shape, dtype))
        self.in_names = in_names
        self.out_names = out_names
        all_names = in_names + out_names
        if partition_name is not None:
            all_names = all_names + [partition_name]

        def _body(*args):
            operands = list(args)
            if partition_name is not None:
                operands.append(partition_id_tensor())
            outs = _bass_exec_p.bind(
                *operands, out_avals=tuple(out_avals),
                in_names=tuple(all_names), out_names=tuple(out_names),
                lowering_input_output_aliases=(),
                sim_require_finite=True, sim_require_nnan=True, nc=nc)
            return tuple(outs)

        devices = jax.devices()[:n_cores]
        self.mesh = Mesh(np.asarray(devices), ("core",))
        n_in = len(in_names) + len(zero_outs)
        self.sharded = jax.jit(
            shard_map(_body, mesh=self.mesh,
                      in_specs=(PartitionSpec("core"),) * n_in,
                      out_specs=(PartitionSpec("core"),) * len(out_names),
                      check_rep=False),
            keep_unused=True)
        self.zero_outs = zero_outs
        self.dev_in = None

    def stage_inputs(self, in_maps):
        from jax.sharding import NamedSharding, PartitionSpec
        sh = NamedSharding(self.mesh, PartitionSpec("core"))
        arrs = []
        for name in self.in_names:
            cat = np.concatenate(
                [np.ascontiguousarray(m[name]) for m in in_maps], axis=0)
            arrs.append(self.jax.device_put(cat, sh))
        for z in self.zero_outs:
            cat = np.zeros((self.n_cores * z.shape[0], *z.shape[1:]), z.dtype)
            arrs.append(self.jax.device_put(cat, sh))
        self.jax.block_until_ready(arrs)
        self.dev_in = arrs

    def run(self):
        return self.sharded(*self.dev_in)

    def results(self):
        outs = self.jax.block_until_ready(self.run())
        res = []
        for c in range(self.n_cores):
            d = {}
            for i, name in enumerate(self.out_names):
                full = np.asarray(outs[i])
                per = full.shape[0] // self.n_cores
                d[name] = full[c * per:(c + 1) * per]
            res.append(d)
        return res


def _posenc():
    pos = np.arange(TD, dtype=np.float32)[:, None]
    div = np.exp(np.arange(0, D, 2, dtype=np.float32)
                 * np.float32(-math.log(10000.0) / D))
    pe = np.zeros((TD, D), np.float32)
    pe[:, 0::2] = np.sin(pos * div)
    pe[:, 1::2] = np.cos(pos * div)
    return pe


def _prep_inputs(inputs):
    f32 = lambda k: np.asarray(inputs[k], np.float32)
    targets = np.asarray(inputs["targets"]).astype(np.int64)
    tlen = np.asarray(inputs["target_lengths"]).astype(np.int64)
    elen = np.asarray(inputs["encoder_output_lengths"]).astype(np.int64)
    enc = f32("encoder_outputs")
    emb_scaled = f32("emb") * np.float32(math.sqrt(float(D)))
    pe = _posenc()

    pet = np.concatenate([pe.T] * BL, axis=1)                # [D, TOK]
    mask01t = (np.arange(P)[:, None] <= np.arange(P)[None, :]
               ).astype(np.float32)                          # [k, q] keep
    ones128 = np.ones((P, P), np.float32)

    shared = {
        "pet": pet, "mask01t": mask01t, "ones128": ones128,
        "w_in": f32("W_in"), "b_in": f32("b_in"),
        "lng": np.stack([f32("ln1_g"), f32("ln2_g"), f32("ln3_g")]),
        "lnb": np.stack([f32("ln1_b"), f32("ln2_b"), f32("ln3_b")]),
        "wq1": f32("Wq1"), "wk1": f32("Wk1"), "wv1": f32("Wv1"),
        "wo1": f32("Wo1"), "wq2": f32("Wq2"), "wk2": f32("Wk2"),
        "wv2": f32("Wv2"), "wo2": f32("Wo2"),
        "bq1": f32("bq1"), "bk1": f32("bk1"), "bv1": f32("bv1"),
        "bo1": f32("bo1"), "bq2": f32("bq2"), "bk2": f32("bk2"),
        "bv2": f32("bv2"), "bo2": f32("bo2"),
        "wff1": f32("W_ff1"), "bff1": f32("b_ff1"),
        "wff2": f32("W_ff2"), "bff2": f32("b_ff2"),
    }

    karange = np.arange(P)
    in_maps = []
    for c in range(NCORES):
        x0t = np.empty((D, TOK), np.float32)
        enct = np.empty((D, ETOK), np.float32)
        selfpad = np.zeros((BL * 2, P), np.float32)
        crosspad = np.zeros((BL * 8, P), np.float32)
        for b in range(BL):
            g = c * BL + b
            x0t[:, b * TD:(b + 1) * TD] = emb_scaled[targets[g]].T
            enct[:, b * TE:(b + 1) * TE] = enc[g].T
            for kt in range(2):
                kpos = kt * P + karange
                selfpad[b * 2 + kt] = np.where(kpos < tlen[g], 0.0, NEG)
            for kt in range(8):
                kpos = kt * P + karange
                crosspad[b * 8 + kt] = np.where(kpos < elen[g], 0.0, NEG)
        m = dict(shared)
        m.update({"x0t": x0t, "enct": enct,
                  "selfpad": selfpad, "crosspad": crosspad})
        in_maps.append(m)
    return shared, in_maps


def _weights_key(shared):
    import hashlib
    h = hashlib.sha256()
    for k in sorted(shared):
        h.update(k.encode())
        h.update(np.ascontiguousarray(shared[k]).tobytes())
    return h.hexdigest()


def _get_runner(shared):
    key = _weights_key(shared)
    if _CACHE.get("key") != key:
        nc = build_nc(NL, weights=shared)
        _CACHE["nc"] = nc
        _CACHE["runner"] = _Runner(nc, NCORES)
        _CACHE["key"] = key
    return _CACHE["runner"]


def kernel(**inputs):
    shared, in_maps = _prep_inputs(inputs)
    runner = _get_runner(shared)
    runner.stage_inputs(in_maps)
    res = runner.results()
    out = np.empty((B, TD, D), np.float32)
    for c in range(NCORES):
        outt = res[c]["outt"]                    # [D, TOK]
        for b in range(BL):
            out[c * BL + b] = outt[:, b * TD:(b + 1) * TD].T
    return out
